# revision 1
# baseline (speedup 1.0000x reference)
"""Distributed GQA attention block for Trainium2 (8 NeuronCores).

Problem: nn_Attention_65927747993826
  x:[2,2048,2048] f32, causal GQA attention, H=32 query heads, G=8 KV groups,
  head_size=64, with q/k/v/out projections and bias.

Sharding (8-way head parallel): core c owns query heads [4c, 4c+4) and KV
group c. Each core computes q/k/v projections for its heads from the full x,
causal flash-attention for its 4 heads, and a partial output projection
through its 256 rows of Wo. The host sums the 8 partial outputs and adds the
bias (a per-feature constant commutes with the partial-sum reduction).

Layouts on chip are feature-major ("transposed"): x^T [E, S] etc., so every
matmul contracts over the partition dim with zero on-chip transposes except
v (PE-transposed). Compute dtype bf16 (f32 accumulate in PSUM).
"""

from contextlib import ExitStack

import numpy as np
import ml_dtypes

import concourse.bass as bass
import concourse.mybir as mybir
import concourse.tile as tile
from concourse import bacc
from concourse.bass import ts, ds
from concourse.bass_utils import run_bass_kernel_spmd
from concourse.masks import make_identity, make_upper_triangular

B, S, E = 2, 2048, 2048
H, G, D = 32, 8, 64
NCORES = 8
HPC = H // NCORES            # query heads per core: 4
FPC = HPC * D                # q features per core: 256
P = 128
KT = E // P                  # 16 contraction tiles over E
NT = S // 512                # 4 token 512-blocks per batch
SCALE = D ** -0.5
F32 = mybir.dt.float32
BF16 = mybir.dt.bfloat16
FA = mybir.ActivationFunctionType
ALU = mybir.AluOpType


def build_nc():
    nc = bacc.Bacc()
    # x_t tiled [B, NT, E, 512]: each (b, n) token-block is a dense 2MB
    # region so the strided per-partition DMA rows stay page-local
    x_t = nc.declare_dram_parameter("x_t", [B, NT, E, 512], BF16, isOutput=False)
    wq = nc.declare_dram_parameter("wq", [E, FPC], BF16, isOutput=False)
    wkv = nc.declare_dram_parameter("wkv", [E, P], BF16, isOutput=False)
    wo = nc.declare_dram_parameter("wo", [FPC, E], BF16, isOutput=False)
    # out tiled [B, KT, NT, 128, 512]: every output DMA is one contiguous
    # 128KB block; the host reassembles
    out = nc.declare_dram_parameter("out", [B, KT, NT, P, 512], BF16, isOutput=True)

    with ExitStack() as ctx:
        tc = ctx.enter_context(tile.TileContext(nc))
        consts = ctx.enter_context(tc.tile_pool(name="consts", bufs=1))
        wpool = ctx.enter_context(tc.tile_pool(name="w", bufs=1))
        xbp = ctx.enter_context(tc.tile_pool(name="xb", bufs=4))
        qkvp = ctx.enter_context(tc.tile_pool(name="qkv", bufs=2))
        ppool = ctx.enter_context(tc.tile_pool(name="probs", bufs=14))
        npool = ctx.enter_context(tc.tile_pool(name="norm", bufs=2))
        opool = ctx.enter_context(tc.tile_pool(name="outsb", bufs=4))
        pp_mm = ctx.enter_context(tc.tile_pool(name="pmm", bufs=2, space="PSUM"))
        pp_sp = ctx.enter_context(tc.tile_pool(name="psp", bufs=2, space="PSUM"))
        pp_acc = ctx.enter_context(tc.tile_pool(name="pacc", bufs=2, space="PSUM"))

        # ---- constants ----
        ident = consts.tile([P, P], BF16)
        make_identity(nc, ident)
        tri = consts.tile([P, P], BF16)  # tri[k, q] = 1 iff q >= k
        make_upper_triangular(nc, tri, val=1.0, diag=True)
        # sel2 row 0 selects partitions 0-63, row 32 selects 64-127 (rows are
        # 32-aligned — the engines reject other base partitions). lhsT of the
        # K=64 broadcast matmul that fans the per-head 1/denom rows out to the
        # 64-partition feature blocks of attnsb.
        sel2 = consts.tile([64, P], BF16)
        nc.gpsimd.memset(sel2, 0.0)
        nc.gpsimd.memset(sel2[0:1, 0:64], 1.0)
        nc.gpsimd.memset(sel2[32:33, 64:128], 1.0)

        # ---- weights (gpsimd DMA queue, parallel to the x loads on sync) ----
        # wq in 4 ko-chunks so the opening q-proj matmuls only wait on the
        # first 256KB instead of the full megabyte
        wq_sb = wpool.tile([P, KT, FPC], BF16)
        wq_r = wq.rearrange("(ko p) m -> p ko m", p=P)
        for ch in range(4):
            nc.gpsimd.dma_start(wq_sb[:, ds(4 * ch, 4), :], wq_r[:, ds(4 * ch, 4), :])
        wkv_sb = wpool.tile([P, KT, P], BF16)
        nc.gpsimd.dma_start(wkv_sb, wkv.rearrange("(ko p) m -> p ko m", p=P))
        wo_sb = wpool.tile([P, 2, E], BF16)
        nc.gpsimd.dma_start(wo_sb, wo.rearrange("(ko p) m -> p ko m", p=P))

        state = {("normed", 0): [0], ("normed", 1): [0]}

        def gen_proj_n(b, n):
            """projection of token-block n for batch b (PE-heavy)."""
            if n == 0:
                # q4: per-head q^T at partitions 0-63, zeros at 64-127 so the
                # scores matmul contracts over a full 128 partitions (keeps
                # the PE in one tile mode — no mode-switch drains).
                q4 = qkvp.tile([P, HPC, S], BF16, tag="q4")
                nc.gpsimd.memset(q4[64:128, :, :], 0.0)
                # k_pad: k^T rows 0-63, zeros 64-127 (scores lhsT)
                k_pad = qkvp.tile([P, S], BF16, tag="kpad")
                nc.gpsimd.memset(k_pad[64:128, :], 0.0)
                # kv: k^T rows 0-63, v^T rows 64-127 (v-transpose source)
                kvsb = qkvp.tile([P, S], BF16, tag="kv")
                # v token-major (+ ones column), PE-transposed per block
                vsb = qkvp.tile([P, S // P, D + 1], BF16, tag="v")
                nc.gpsimd.memset(vsb[:, :, D : D + 1], 1.0)
                # attnsb holds UNNORMALIZED attn until deferred normalize
                attnsb = qkvp.tile([P, 2, S], BF16, tag="attn")
                state[b] = (q4, k_pad, kvsb, vsb, attnsb)
            q4, k_pad, kvsb, vsb, attnsb = state[b]
            # two half-tiles with separate DMAs: the first 8 k-tiles of
            # matmul only wait on the first half's DMA. The very first
            # block is DMA'd in quarters so the opening matmul can start
            # as early as possible.
            xh = []
            nchunk = 4 if (b == 0 and n == 0) else 1
            # very first block: spread the quarter DMAs across four engine
            # queues so the transfers run in parallel instead of serializing
            # on the sync queue
            qeng = [nc.sync, nc.scalar, nc.sync, nc.scalar]
            for g in range(2):
                xb = xbp.tile([P, KT // 2, 512], BF16)
                for q_ in range(nchunk):
                    eng = qeng[q_] if nchunk > 1 else nc.sync
                    eng.dma_start(
                        xb[:, ds(q_ * (8 // nchunk), 8 // nchunk), :],
                        x_t[
                            b, n,
                            ds(1024 * g + q_ * (1024 // nchunk), 1024 // nchunk),
                            :,
                        ].rearrange("(ko p) s -> p ko s", p=P),
                    )
                xh.append(xb)
            for m in range(3):
                ps = pp_mm.tile([P, 512], F32, tag="mm")
                for k in range(KT):
                    lhsT = wq_sb[:, k, ts(m, P)] if m < 2 else wkv_sb[:, k, :]
                    nc.tensor.matmul(
                        ps,
                        lhsT,
                        xh[k // 8][:, k % 8, :],
                        start=(k == 0),
                        stop=(k == KT - 1),
                    )
                if m < 2:
                    nc.vector.tensor_copy(q4[0:64, 2 * m, ts(n, 512)], ps[0:64, :])
                    nc.vector.tensor_copy(
                        q4[0:64, 2 * m + 1, ts(n, 512)], ps[64:128, :]
                    )
                else:
                    nc.vector.tensor_copy(kvsb[:, ts(n, 512)], ps)
                    nc.vector.tensor_copy(k_pad[0:64, ts(n, 512)], ps[0:64, :])
                yield
            # v transposes for this token-block, done per-block so a proj
            # generator that finishes block n has also produced vsb block n —
            # lets proj(1)'s later blocks run as phase-3 filler without the
            # attention racing ahead of its vsb inputs in the PE queue
            for st in range(4 * n, 4 * n + 4):
                tp = pp_mm.tile([P, P], BF16, tag="mm")
                nc.tensor.transpose(tp, kvsb[:, ts(st, P)], ident)
                nc.vector.tensor_copy(vsb[:, st, 0:D], tp[:, 64:128])
            yield

        def gen_att_qt(b, qt):
            """causal attention for q-block qt of batch b (ACT-heavy: exp).

            softmax denominators come for free from the ones-column of v_aug
            (row 64 of each accumulator); normalization is deferred to a
            batched approximate reciprocal + selector-matmul broadcast per
            q-block.
            """
            q4, k_pad, kvsb, vsb, attnsb = state[b]
            if True:
                # dq[32*(h%2), 512*(h//2) + q] = softmax denominator of head
                # h at query q (rows 32-aligned). The 1.0 background keeps the
                # reciprocal finite on the unused rows so the selector matmul
                # contracts only finite values.
                dq = npool.tile([64, 1024], F32, tag="den")
                nc.gpsimd.memset(dq, 1.0)
                nkt = 4 * (qt + 1)
                for h in range(HPC):
                    acc = pp_acc.tile([D + 1, 512], F32, tag="acc")
                    nfull = 4 * qt
                    # 1) diagonal scores first: their exp+mask chains get
                    #    maximum slack before their attnV consumers issue
                    #    last. t=0 gets its own PSUM tile; t=1..3 pack into a
                    #    second one (bank-aligned at 0/512/768) so ONE wide
                    #    exp covers all three — fewer ACTIVATEs on the
                    #    engine that paces the attention phase. The packed
                    #    ACTIVATE also exps the stale gap cols 384..512;
                    #    harmless (scaled stale PSUM stays well below f32
                    #    overflow, and no consumer reads those pr cols).
                    dof = {1: 0, 2: 512, 3: 768}
                    spA = pp_sp.tile([P, 1024], F32, tag="sp")
                    nc.tensor.matmul(
                        spA[:, 0:512],
                        k_pad[:, ts(nfull, P)],
                        q4[:, h, ts(qt, 512)],
                        start=True,
                        stop=True,
                    )
                    prA = ppool.tile([P, 1024], BF16, tag="pr")
                    nc.scalar.activation(
                        prA[:, 0:512], spA[:, 0:512], FA.Exp, scale=SCALE
                    )
                    nc.vector.tensor_tensor(
                        prA[:, 0:P], prA[:, 0:P], tri, ALU.mult
                    )
                    spB = pp_sp.tile([P, 1024], F32, tag="sp")
                    for t in range(1, 4):
                        kt = nfull + t
                        w_ = 512 - t * P
                        nc.tensor.matmul(
                            spB[:, ds(dof[t], w_)],
                            k_pad[:, ts(kt, P)],
                            q4[:, h, ds(512 * qt + t * P, w_)],
                            start=True,
                            stop=True,
                        )
                    prB = ppool.tile([P, 1024], BF16, tag="pr")
                    nc.scalar.activation(
                        prB[:, 0:896], spB[:, 0:896], FA.Exp, scale=SCALE
                    )
                    for t in range(1, 4):
                        nc.vector.tensor_tensor(
                            prB[:, ds(dof[t], P)], prB[:, ds(dof[t], P)],
                            tri, ALU.mult,
                        )
                    yield
                    # 2) full (off-diagonal) k-tiles, paired two per PSUM tile
                    #    so one exp ACTIVATE covers 1024 columns; attnV follows
                    #    each pair immediately (no mask on its path)
                    first = True
                    for kp in range(0, nfull, 2):
                        sp = pp_sp.tile([P, 1024], F32, tag="sp")
                        for j in range(2):
                            kt = kp + j
                            nc.tensor.matmul(
                                sp[:, ts(j, 512)],
                                k_pad[:, ts(kt, P)],
                                q4[:, h, ts(qt, 512)],
                                start=True,
                                stop=True,
                            )
                        pr = ppool.tile([P, 1024], BF16, tag="pr")
                        nc.scalar.activation(pr, sp, FA.Exp, scale=SCALE)
                        for j in range(2):
                            nc.tensor.matmul(
                                acc,
                                vsb[:, kp + j, :],
                                pr[:, ts(j, 512)],
                                start=first,
                                stop=False,
                            )
                            first = False
                        if kp % 4 == 2:
                            yield
                    # 3) diagonal attnV last
                    for t in range(4):
                        kt = nfull + t
                        off = t * P
                        w_ = 512 - off
                        src = prA[:, ds(off, w_)] if t == 0 else prB[:, ds(dof[t], w_)]
                        nc.tensor.matmul(
                            acc[:, ds(off, w_)],
                            vsb[:, kt, :],
                            src,
                            start=first,
                            stop=(t == 3),
                        )
                        first = False
                    yield
                    # evacuate unnormalized attn + denominator row (head h's
                    # denominators land at partition 32h for broadcast later)
                    # denominator row first: it gates the reciprocal chain,
                    # while the attn-block copy can overlap the reciprocal
                    dst = attnsb[64 * (h % 2) : 64 * (h % 2) + 64, h // 2, ts(qt, 512)]
                    nc.vector.tensor_copy(
                        dq[32 * (h % 2) : 32 * (h % 2) + 1, ds(512 * (h // 2), 512)],
                        acc[D : D + 1, :],
                    )
                    nc.vector.tensor_copy(dst, acc[0:D, :])
                    yield
                # filler slots: the normalize below is a serial DVE chain and
                # the bc matmul behind it stalls the in-order PE queue; these
                # yields let the interleaver emit filler bursts into exactly
                # that queue position
                yield
                yield
                yield
                # deferred softmax normalization: one fast approximate
                # reciprocal (18-bit, plenty for bf16 data), a bf16 cast,
                # then one K=64 selector matmul (2 PSUM-bank halves)
                # broadcasts both feature blocks' 1/denom rows across
                # partitions, and two multiplies apply them
                rec = npool.tile([64, 1024], F32, tag="rec")
                nc.vector.reciprocal_approx_fast(rec, dq)
                recb = npool.tile([64, 1024], BF16, tag="recb")
                nc.vector.tensor_copy(recb, rec)
                bc = pp_sp.tile([P, 1024], F32, tag="sp")
                for kk in range(2):
                    nc.tensor.matmul(
                        bc[:, ts(kk, 512)], sel2, recb[:, ts(kk, 512)],
                        start=True, stop=True,
                    )
                for kk in range(2):
                    dst = attnsb[:, kk, ts(qt, 512)]
                    nc.vector.tensor_tensor(dst, dst, bc[:, ts(kk, 512)], ALU.mult)
                state[("normed", b)][0] += 1
                yield

        def gen_outproj(b, use_act, ns=None):
            """partial output projection for batch b (PE-heavy).

            n-outer so the last q-block's softmax-normalize latency is hidden
            behind the first 3 n-blocks' matmuls. use_act alternates the PSUM
            evacuation onto ScalarE only when no attention phase is keeping
            ScalarE saturated with exps.

            Gated on the normalize counter: the normalize path now contains a
            PE matmul (the 1/denom broadcast), so emitting an outproj matmul
            that waits on q-block n's normalize BEFORE that broadcast matmul
            is emitted would deadlock the in-order PE queue.
            """
            attnsb = state[b][4]
            for n in ns if ns is not None else range(NT):
                while state[("normed", b)][0] <= n:
                    yield
                for m in range(KT):
                    po = pp_mm.tile([P, 512], F32, tag="mm")
                    for kk in range(2):
                        nc.tensor.matmul(
                            po,
                            wo_sb[:, kk, ts(m, P)],
                            attnsb[:, kk, ts(n, 512)],
                            start=(kk == 0),
                            stop=(kk == 1),
                        )
                    osb = opool.tile([P, 512], BF16)
                    # use_act None: never touch ScalarE (it paces the
                    # attention phase this generator fills); True: quiet-tail
                    # block, lean on ScalarE to relieve DVE
                    if use_act is True and m % 3 != 0:
                        nc.scalar.copy(osb, po)
                    elif use_act is False and m % 3 == 2:
                        nc.scalar.copy(osb, po)
                    else:
                        nc.vector.tensor_copy(osb, po)
                    # final block: alternate DMA queues so the closing
                    # transfers drain in parallel instead of serializing on
                    # the sync queue after the last matmul
                    deng = nc.scalar if (b == 1 and n == 3 and m % 2 == 1) else nc.sync
                    deng.dma_start(out[b, m, n, :, :], osb)
                    if m % 4 == 3:
                        yield

        def run_all(gen):
            for _ in gen:
                pass

        def interleave(pairs):
            """pairs: list of [gen, steps_per_round]. Round-robin with ratios
            so the PE-filler generator is spread across the whole phase."""
            pairs = [[g, r] for g, r in pairs]
            while pairs:
                for gr in pairs[:]:
                    try:
                        for _ in range(gr[1]):
                            next(gr[0])
                    except StopIteration:
                        pairs.remove(gr)

        def delayed(gen, k):
            for _ in range(k):
                yield
            yield from gen

        def chain(gens):
            for g in gens:
                yield from g

        # Pipeline the two batches so PE-heavy projection work fills the PE
        # bubbles of the ACT(exp)-bound attention phases; out-projections
        # enter a phase early, delayed so their first matmuls trail the
        # q-block normalizes they depend on in the in-order PE stream.
        op0 = gen_outproj(0, None)
        # both ops' bulk runs inside exp-saturated attention phases -> keep
        # their evacuations off the scalar engine entirely; only the final
        # n=3 block (the quiet tail) borrows ScalarE
        op1 = gen_outproj(1, None, ns=[0, 1, 2])
        op1t = gen_outproj(1, True, ns=[3])
        proj = lambda b: chain([gen_proj_n(b, n) for n in range(NT)])
        att = lambda b: chain([gen_att_qt(b, qt) for qt in range(NT)])
        run_all(proj(0))
        interleave([(att(0), 4), (proj(1), 1), (op0, 1)])
        interleave([(op0, 1), (att(1), 4), (op1, 2)])
        run_all(op1)
        run_all(op1t)
    return nc


BF = ml_dtypes.bfloat16


def make_in_maps(x, Wq, Wk, Wv, Wo):
    # [B, S, E] -> [B, NT, E, 512] (token-block-tiled, feature-major)
    x_t = np.ascontiguousarray(
        np.transpose(
            np.asarray(x, np.float32).reshape(B, NT, 512, E), (0, 1, 3, 2)
        )
    ).astype(BF)
    Wq = np.asarray(Wq, np.float32)
    Wk = np.asarray(Wk, np.float32)
    Wv = np.asarray(Wv, np.float32)
    Wo = np.asarray(Wo, np.float32)
    in_maps = []
    for c in range(NCORES):
        wq_sh = np.ascontiguousarray(Wq[:, FPC * c : FPC * (c + 1)]).astype(BF)
        wkv_sh = np.concatenate(
            [Wk[:, D * c : D * (c + 1)], Wv[:, D * c : D * (c + 1)]], axis=1
        ).astype(BF)
        wo_sh = np.ascontiguousarray(Wo[FPC * c : FPC * (c + 1), :]).astype(BF)
        in_maps.append({"x_t": x_t, "wq": wq_sh, "wkv": wkv_sh, "wo": wo_sh})
    return in_maps


_NC_CACHE = {}


def get_nc():
    if "nc" not in _NC_CACHE:
        nc = build_nc()
        nc.compile()
        _NC_CACHE["nc"] = nc
    return _NC_CACHE["nc"]


def kernel(x, Wq, Wk, Wv, Wo, bo, mask=None, **_ignored):
    nc = get_nc()
    in_maps = make_in_maps(x, Wq, Wk, Wv, Wo)
    res = run_bass_kernel_spmd(nc, in_maps, list(range(NCORES)))
    total = np.zeros((B, KT, NT, P, 512), np.float32)
    for c in range(NCORES):
        total += np.asarray(res.results[c]["out"], np.float32)
    # [B, KT, NT, 128, 512] -> [B, S, E]: feature = m*128+p, token = n*512+s
    full = np.transpose(total, (0, 2, 4, 1, 3)).reshape(B, S, E)
    full = full + np.asarray(bo, np.float32)[None, None, :]
    return np.ascontiguousarray(full)



# revision 10
# speedup vs baseline: 1.1050x; 1.1050x over previous
"""Distributed GQA attention block for Trainium2 (8 NeuronCores).

Problem: nn_Attention_65927747993826
  x:[2,2048,2048] f32, causal GQA attention, H=32 query heads, G=8 KV groups,
  head_size=64, with q/k/v/out projections and bias.

Sharding (8-way head parallel): core c owns query heads [4c, 4c+4) and KV
group c. Each core computes q/k/v projections for its heads from the full x,
causal flash-attention for its 4 heads, and a partial output projection
through its 256 rows of Wo. The host sums the 8 partial outputs and adds the
bias (a per-feature constant commutes with the partial-sum reduction).

Layouts on chip are feature-major ("transposed"): x^T [E, S] etc., so every
matmul contracts over the partition dim with zero on-chip transposes except
v (PE-transposed). Compute dtype bf16 (f32 accumulate in PSUM).
"""

from contextlib import ExitStack

import numpy as np
import ml_dtypes

import concourse.bass as bass
import concourse.mybir as mybir
import concourse.tile as tile
from concourse import bacc
from concourse.bass import ts, ds
from concourse.bass_utils import run_bass_kernel_spmd
from concourse.masks import make_identity, make_upper_triangular

B, S, E = 2, 2048, 2048
H, G, D = 32, 8, 64
NCORES = 8
HPC = H // NCORES            # query heads per core: 4
FPC = HPC * D                # q features per core: 256
P = 128
KT = E // P                  # 16 contraction tiles over E
NT = S // 512                # 4 token 512-blocks per batch
SCALE = D ** -0.5
F32 = mybir.dt.float32
BF16 = mybir.dt.bfloat16
FA = mybir.ActivationFunctionType
ALU = mybir.AluOpType


def build_nc():
    nc = bacc.Bacc()
    # x_t tiled [B, NT, E, 512]: each (b, n) token-block is a dense 2MB
    # region so the strided per-partition DMA rows stay page-local
    x_t = nc.declare_dram_parameter("x_t", [B, NT, E, 512], BF16, isOutput=False)
    wq = nc.declare_dram_parameter("wq", [E, FPC], BF16, isOutput=False)
    wkv = nc.declare_dram_parameter("wkv", [E, P], BF16, isOutput=False)
    wo = nc.declare_dram_parameter("wo", [FPC, E], BF16, isOutput=False)
    # out tiled [B, KT, NT, 128, 512]: every output DMA is one contiguous
    # 128KB block; the host reassembles
    out = nc.declare_dram_parameter("out", [B, KT, NT, P, 512], BF16, isOutput=True)

    with ExitStack() as ctx:
        tc = ctx.enter_context(tile.TileContext(nc))
        consts = ctx.enter_context(tc.tile_pool(name="consts", bufs=1))
        wpool = ctx.enter_context(tc.tile_pool(name="w", bufs=1))
        xbp = ctx.enter_context(tc.tile_pool(name="xb", bufs=4))
        qkvp = ctx.enter_context(tc.tile_pool(name="qkv", bufs=2))
        ppool = ctx.enter_context(tc.tile_pool(name="probs", bufs=14))
        npool = ctx.enter_context(tc.tile_pool(name="norm", bufs=2))
        opool = ctx.enter_context(tc.tile_pool(name="outsb", bufs=4))
        pp_mm = ctx.enter_context(tc.tile_pool(name="pmm", bufs=2, space="PSUM"))
        pp_sp = ctx.enter_context(tc.tile_pool(name="psp", bufs=2, space="PSUM"))
        pp_acc = ctx.enter_context(tc.tile_pool(name="pacc", bufs=2, space="PSUM"))

        # ---- constants ----
        ident = consts.tile([P, P], BF16)
        make_identity(nc, ident)
        tri = consts.tile([P, P], BF16)  # tri[k, q] = 1 iff q >= k
        make_upper_triangular(nc, tri, val=1.0, diag=True)
        # sel2 row 0 selects partitions 0-63, row 32 selects 64-127 (rows are
        # 32-aligned — the engines reject other base partitions). lhsT of the
        # K=64 broadcast matmul that fans the per-head 1/denom rows out to the
        # 64-partition feature blocks of attnsb.
        sel2 = consts.tile([64, P], BF16)
        nc.gpsimd.memset(sel2, 0.0)
        nc.gpsimd.memset(sel2[0:1, 0:64], 1.0)
        nc.gpsimd.memset(sel2[32:33, 64:128], 1.0)

        # ---- weights (gpsimd DMA queue, parallel to the x loads on sync) ----
        # wq in 4 ko-chunks so the opening q-proj matmuls only wait on the
        # first 256KB instead of the full megabyte
        wq_sb = wpool.tile([P, KT, FPC], BF16)
        wq_r = wq.rearrange("(ko p) m -> p ko m", p=P)
        for ch in range(4):
            nc.gpsimd.dma_start(wq_sb[:, ds(4 * ch, 4), :], wq_r[:, ds(4 * ch, 4), :])
        wkv_sb = wpool.tile([P, KT, P], BF16)
        nc.gpsimd.dma_start(wkv_sb, wkv.rearrange("(ko p) m -> p ko m", p=P))
        wo_sb = wpool.tile([P, 2, E], BF16)
        nc.gpsimd.dma_start(wo_sb, wo.rearrange("(ko p) m -> p ko m", p=P))

        state = {("normed", 0): [0], ("normed", 1): [0]}

        def gen_proj_n(b, n):
            """projection of token-block n for batch b (PE-heavy)."""
            if n == 0:
                # q4: per-head q^T at partitions 0-63, zeros at 64-127 so the
                # scores matmul contracts over a full 128 partitions (keeps
                # the PE in one tile mode — no mode-switch drains).
                q4 = qkvp.tile([P, HPC, S], BF16, tag="q4")
                nc.gpsimd.memset(q4[64:128, :, :], 0.0)
                # k_pad: k^T rows 0-63, zeros 64-127 (scores lhsT)
                k_pad = qkvp.tile([P, S], BF16, tag="kpad")
                nc.gpsimd.memset(k_pad[64:128, :], 0.0)
                # kv: k^T rows 0-63, v^T rows 64-127 (v-transpose source)
                kvsb = qkvp.tile([P, S], BF16, tag="kv")
                # v token-major (+ ones column), PE-transposed per block
                vsb = qkvp.tile([P, S // P, D + 1], BF16, tag="v")
                nc.gpsimd.memset(vsb[:, :, D : D + 1], 1.0)
                # attnsb holds UNNORMALIZED attn until deferred normalize
                attnsb = qkvp.tile([P, 2, S], BF16, tag="attn")
                state[b] = (q4, k_pad, kvsb, vsb, attnsb)
            q4, k_pad, kvsb, vsb, attnsb = state[b]
            # two half-tiles with separate DMAs: the first 8 k-tiles of
            # matmul only wait on the first half's DMA. The very first
            # block is DMA'd in quarters so the opening matmul can start
            # as early as possible.
            xh = []
            nchunk = 4 if (b == 0 and n == 0) else 1
            # very first block: spread the quarter DMAs across the two
            # hwdge engine queues so the transfers run in parallel instead
            # of serializing on the sync queue
            qeng = [nc.sync, nc.scalar, nc.sync, nc.scalar]
            for g in range(2):
                xb = xbp.tile([P, KT // 2, 512], BF16)
                for q_ in range(nchunk):
                    eng = qeng[q_] if nchunk > 1 else nc.sync
                    eng.dma_start(
                        xb[:, ds(q_ * (8 // nchunk), 8 // nchunk), :],
                        x_t[
                            b, n,
                            ds(1024 * g + q_ * (1024 // nchunk), 1024 // nchunk),
                            :,
                        ].rearrange("(ko p) s -> p ko s", p=P),
                    )
                xh.append(xb)
            for m in range(3):
                ps = pp_mm.tile([P, 512], F32, tag="mm")
                for k in range(KT):
                    lhsT = wq_sb[:, k, ts(m, P)] if m < 2 else wkv_sb[:, k, :]
                    nc.tensor.matmul(
                        ps,
                        lhsT,
                        xh[k // 8][:, k % 8, :],
                        start=(k == 0),
                        stop=(k == KT - 1),
                    )
                if m < 2:
                    nc.vector.tensor_copy(q4[0:64, 2 * m, ts(n, 512)], ps[0:64, :])
                    nc.vector.tensor_copy(
                        q4[0:64, 2 * m + 1, ts(n, 512)], ps[64:128, :]
                    )
                else:
                    nc.vector.tensor_copy(kvsb[:, ts(n, 512)], ps)
                    nc.vector.tensor_copy(k_pad[0:64, ts(n, 512)], ps[0:64, :])
                yield
            # v transposes for this token-block, done per-block so a proj
            # generator that finishes block n has also produced vsb block n —
            # lets proj(1)'s later blocks run as phase-3 filler without the
            # attention racing ahead of its vsb inputs in the PE queue
            for st in range(4 * n, 4 * n + 4):
                tp = pp_mm.tile([P, P], BF16, tag="mm")
                nc.tensor.transpose(tp, kvsb[:, ts(st, P)], ident)
                nc.vector.tensor_copy(vsb[:, st, 0:D], tp[:, 64:128])
            yield

        def gen_att_qt(b, qt):
            """causal attention for q-block qt of batch b (ACT-heavy: exp).

            softmax denominators come for free from the ones-column of v_aug
            (row 64 of each accumulator); per-head approximate reciprocals
            run straight from PSUM, and the rest of the normalization (bf16
            cast + selector-matmul broadcast + multiplies) is DEFERRED into
            the next q-block's instruction stream (gen_norm), so the bc
            matmul never stalls the in-order PE queue waiting on the DVE
            reciprocal chain.
            """
            q4, k_pad, kvsb, vsb, attnsb = state[b]
            if True:
                # dq[32*(h%2), 512*(h//2) + q] = 1/denominator of head h at
                # query q (rows 32-aligned). The 1.0 background keeps the
                # selector matmul contracting only finite values.
                dq = npool.tile([64, 1024], F32, tag="den")
                nc.gpsimd.memset(dq, 1.0)
                nkt = 4 * (qt + 1)
                for h in range(HPC):
                    acc = pp_acc.tile([D + 1, 512], F32, tag="acc")
                    nfull = 4 * qt
                    # 1) diagonal scores first: their exp+mask chains get
                    #    maximum slack before their attnV consumers issue
                    #    last. t=0 gets its own PSUM tile; t=1..3 pack into a
                    #    second one (bank-aligned at 0/512/768) so ONE wide
                    #    exp covers all three — fewer ACTIVATEs on the
                    #    engine that paces the attention phase. The packed
                    #    ACTIVATE also exps the stale gap cols 384..512;
                    #    harmless (scaled stale PSUM stays well below f32
                    #    overflow, and no consumer reads those pr cols).
                    dof = {1: 0, 2: 512, 3: 768}
                    spA = pp_sp.tile([P, 1024], F32, tag="sp")
                    nc.tensor.matmul(
                        spA[:, 0:512],
                        k_pad[:, ts(nfull, P)],
                        q4[:, h, ts(qt, 512)],
                        start=True,
                        stop=True,
                    )
                    prA = ppool.tile([P, 1024], BF16, tag="pr")
                    nc.scalar.activation(
                        prA[:, 0:512], spA[:, 0:512], FA.Exp, scale=SCALE
                    )
                    nc.vector.tensor_tensor(
                        prA[:, 0:P], prA[:, 0:P], tri, ALU.mult
                    )
                    spB = pp_sp.tile([P, 1024], F32, tag="sp")
                    for t in range(1, 4):
                        kt = nfull + t
                        w_ = 512 - t * P
                        nc.tensor.matmul(
                            spB[:, ds(dof[t], w_)],
                            k_pad[:, ts(kt, P)],
                            q4[:, h, ds(512 * qt + t * P, w_)],
                            start=True,
                            stop=True,
                        )
                    prB = ppool.tile([P, 1024], BF16, tag="pr")
                    nc.scalar.activation(
                        prB[:, 0:896], spB[:, 0:896], FA.Exp, scale=SCALE
                    )
                    for t in range(1, 4):
                        nc.vector.tensor_tensor(
                            prB[:, ds(dof[t], P)], prB[:, ds(dof[t], P)],
                            tri, ALU.mult,
                        )
                    yield
                    # 2) full (off-diagonal) k-tiles, paired two per PSUM tile
                    #    so one exp ACTIVATE covers 1024 columns; attnV follows
                    #    each pair immediately (no mask on its path)
                    first = True
                    for kp in range(0, nfull, 2):
                        sp = pp_sp.tile([P, 1024], F32, tag="sp")
                        for j in range(2):
                            kt = kp + j
                            nc.tensor.matmul(
                                sp[:, ts(j, 512)],
                                k_pad[:, ts(kt, P)],
                                q4[:, h, ts(qt, 512)],
                                start=True,
                                stop=True,
                            )
                        pr = ppool.tile([P, 1024], BF16, tag="pr")
                        nc.scalar.activation(pr, sp, FA.Exp, scale=SCALE)
                        for j in range(2):
                            nc.tensor.matmul(
                                acc,
                                vsb[:, kp + j, :],
                                pr[:, ts(j, 512)],
                                start=first,
                                stop=False,
                            )
                            first = False
                        if kp % 4 == 2:
                            yield
                    # 3) diagonal attnV last
                    for t in range(4):
                        kt = nfull + t
                        off = t * P
                        w_ = 512 - off
                        src = prA[:, ds(off, w_)] if t == 0 else prB[:, ds(dof[t], w_)]
                        nc.tensor.matmul(
                            acc[:, ds(off, w_)],
                            vsb[:, kt, :],
                            src,
                            start=first,
                            stop=(t == 3),
                        )
                        first = False
                    yield
                    # evacuate unnormalized attn + denominator row (head h's
                    # denominators land at partition 32(h%2) for broadcast
                    # later). Denominator row first: it gates the deferred
                    # normalize chain.
                    dst = attnsb[64 * (h % 2) : 64 * (h % 2) + 64, h // 2, ts(qt, 512)]
                    nc.vector.tensor_copy(
                        dq[32 * (h % 2) : 32 * (h % 2) + 1, ds(512 * (h // 2), 512)],
                        acc[D : D + 1, :],
                    )
                    nc.vector.tensor_copy(dst, acc[0:D, :])
                    yield
                state[("pending_norm", b)] = gen_norm(b, qt, dq)

        def gen_norm(b, qt, dq):
            """deferred softmax normalization for q-block qt: bf16 cast of
            the 1/denom rows, one K=64 selector matmul (2 PSUM-bank halves)
            broadcasting both feature blocks' rows across partitions, and
            two multiplies. Emitted a few steps into the NEXT q-block so the
            bc matmul queues behind fresh PE work while the cast drains."""
            attnsb = state[b][4]
            rec = npool.tile([64, 1024], F32, tag="rec")
            nc.vector.reciprocal_approx_fast(rec, dq)
            recb = npool.tile([64, 1024], BF16, tag="recb")
            nc.vector.tensor_copy(recb, rec)
            yield
            bc = pp_sp.tile([P, 1024], F32, tag="sp")
            for kk in range(2):
                nc.tensor.matmul(
                    bc[:, ts(kk, 512)], sel2, recb[:, ts(kk, 512)],
                    start=True, stop=True,
                )
            yield
            for kk in range(2):
                dst = attnsb[:, kk, ts(qt, 512)]
                nc.vector.tensor_tensor(dst, dst, bc[:, ts(kk, 512)], ALU.mult)
            state[("normed", b)][0] += 1
            yield

        def gen_outproj(b, use_act, ns=None):
            """partial output projection for batch b (PE-heavy).

            n-outer so the last q-block's softmax-normalize latency is hidden
            behind the first 3 n-blocks' matmuls. use_act alternates the PSUM
            evacuation onto ScalarE only when no attention phase is keeping
            ScalarE saturated with exps.

            Gated on the normalize counter: the normalize path now contains a
            PE matmul (the 1/denom broadcast), so emitting an outproj matmul
            that waits on q-block n's normalize BEFORE that broadcast matmul
            is emitted would deadlock the in-order PE queue.
            """
            attnsb = state[b][4]
            for n in ns if ns is not None else range(NT):
                while state[("normed", b)][0] <= n:
                    yield
                for m in range(KT):
                    po = pp_mm.tile([P, 512], F32, tag="mm")
                    for kk in range(2):
                        nc.tensor.matmul(
                            po,
                            wo_sb[:, kk, ts(m, P)],
                            attnsb[:, kk, ts(n, 512)],
                            start=(kk == 0),
                            stop=(kk == 1),
                        )
                    osb = opool.tile([P, 512], BF16)
                    # use_act None: never touch ScalarE (it paces the
                    # attention phase this generator fills); True: quiet-tail
                    # block, lean on ScalarE to relieve DVE
                    if use_act is True and m % 3 != 0:
                        nc.scalar.copy(osb, po)
                    elif use_act is False and m % 3 == 2:
                        nc.scalar.copy(osb, po)
                    else:
                        nc.vector.tensor_copy(osb, po)
                    # final block: alternate DMA queues so the closing
                    # transfers drain in parallel instead of serializing on
                    # the sync queue after the last matmul
                    deng = nc.scalar if (b == 1 and n == 3 and m % 2 == 1) else nc.sync
                    deng.dma_start(out[b, m, n, :, :], osb)
                    if m % 4 == 3:
                        yield

        def run_all(gen):
            for _ in gen:
                pass

        def interleave(pairs):
            """pairs: list of [gen, steps_per_round]. Round-robin with ratios
            so the PE-filler generator is spread across the whole phase."""
            pairs = [[g, r] for g, r in pairs]
            while pairs:
                for gr in pairs[:]:
                    try:
                        for _ in range(gr[1]):
                            next(gr[0])
                    except StopIteration:
                        pairs.remove(gr)

        def delayed(gen, k):
            for _ in range(k):
                yield
            yield from gen

        def chain(gens):
            for g in gens:
                yield from g

        def att_batch(b):
            """attention for all q-blocks of batch b, draining each block's
            deferred normalize a few steps into the NEXT block (so the bc
            matmul hides behind fresh scores/attnV work), and the final one
            with interleaver turns between its pieces."""
            for qt in range(NT):
                g = gen_att_qt(b, qt)
                steps = 0
                for _ in g:
                    yield
                    steps += 1
                    if steps == 5 and ("pending_norm", b) in state:
                        for _ in state.pop(("pending_norm", b)):
                            yield
            yield
            for _ in state.pop(("pending_norm", b)):
                yield

        # Pipeline the two batches so PE-heavy projection work fills the PE
        # bubbles of the ACT(exp)-bound attention phases; out-projections
        # enter a phase early, delayed so their first matmuls trail the
        # q-block normalizes they depend on in the in-order PE stream.
        # op0 is split so batch-0 outproj filler survives into the batch-1
        # attention phase (its normalize windows need PE work queued behind
        # the bc matmuls).
        op0a = gen_outproj(0, None, ns=[0, 1])
        op0b = gen_outproj(0, None, ns=[2, 3])
        # both ops' bulk runs inside exp-saturated attention phases -> keep
        # their evacuations off the scalar engine entirely; only the final
        # n=3 block (the quiet tail) borrows ScalarE
        op1 = gen_outproj(1, None, ns=[0, 1, 2])
        op1t = gen_outproj(1, True, ns=[3])
        proj = lambda b: chain([gen_proj_n(b, n) for n in range(NT)])
        run_all(proj(0))
        interleave([(att_batch(0), 4), (proj(1), 1), (op0a, 1)])
        interleave([(op0b, 1), (att_batch(1), 4), (op1, 1)])
        run_all(op1)
        run_all(op1t)
    return nc


BF = ml_dtypes.bfloat16


def make_in_maps(x, Wq, Wk, Wv, Wo):
    # [B, S, E] -> [B, NT, E, 512] (token-block-tiled, feature-major)
    x_t = np.ascontiguousarray(
        np.transpose(
            np.asarray(x, np.float32).reshape(B, NT, 512, E), (0, 1, 3, 2)
        )
    ).astype(BF)
    Wq = np.asarray(Wq, np.float32)
    Wk = np.asarray(Wk, np.float32)
    Wv = np.asarray(Wv, np.float32)
    Wo = np.asarray(Wo, np.float32)
    in_maps = []
    for c in range(NCORES):
        wq_sh = np.ascontiguousarray(Wq[:, FPC * c : FPC * (c + 1)]).astype(BF)
        wkv_sh = np.concatenate(
            [Wk[:, D * c : D * (c + 1)], Wv[:, D * c : D * (c + 1)]], axis=1
        ).astype(BF)
        wo_sh = np.ascontiguousarray(Wo[FPC * c : FPC * (c + 1), :]).astype(BF)
        in_maps.append({"x_t": x_t, "wq": wq_sh, "wkv": wkv_sh, "wo": wo_sh})
    return in_maps


_NC_CACHE = {}


def get_nc():
    if "nc" not in _NC_CACHE:
        nc = build_nc()
        nc.compile()
        _NC_CACHE["nc"] = nc
    return _NC_CACHE["nc"]


def kernel(x, Wq, Wk, Wv, Wo, bo, mask=None, **_ignored):
    nc = get_nc()
    in_maps = make_in_maps(x, Wq, Wk, Wv, Wo)
    res = run_bass_kernel_spmd(nc, in_maps, list(range(NCORES)))
    total = np.zeros((B, KT, NT, P, 512), np.float32)
    for c in range(NCORES):
        total += np.asarray(res.results[c]["out"], np.float32)
    # [B, KT, NT, 128, 512] -> [B, S, E]: feature = m*128+p, token = n*512+s
    full = np.transpose(total, (0, 2, 4, 1, 3)).reshape(B, S, E)
    full = full + np.asarray(bo, np.float32)[None, None, :]
    return np.ascontiguousarray(full)



# revision 14
# speedup vs baseline: 1.1318x; 1.0242x over previous
"""Distributed GQA attention block for Trainium2 (8 NeuronCores).

Problem: nn_Attention_65927747993826
  x:[2,2048,2048] f32, causal GQA attention, H=32 query heads, G=8 KV groups,
  head_size=64, with q/k/v/out projections and bias.

Sharding (8-way head parallel): core c owns query heads [4c, 4c+4) and KV
group c. Each core computes q/k/v projections for its heads from the full x,
causal flash-attention for its 4 heads, and a partial output projection
through its 256 rows of Wo. The host sums the 8 partial outputs and adds the
bias (a per-feature constant commutes with the partial-sum reduction).

Layouts on chip are feature-major ("transposed"): x^T [E, S] etc., so every
matmul contracts over the partition dim with zero on-chip transposes except
v (PE-transposed). Compute dtype bf16 (f32 accumulate in PSUM).
"""

from contextlib import ExitStack

import numpy as np
import ml_dtypes

import concourse.bass as bass
import concourse.mybir as mybir
import concourse.tile as tile
from concourse import bacc
from concourse.bass import ts, ds
from concourse.bass_utils import run_bass_kernel_spmd
from concourse.masks import make_identity, make_upper_triangular

B, S, E = 2, 2048, 2048
H, G, D = 32, 8, 64
NCORES = 8
HPC = H // NCORES            # query heads per core: 4
FPC = HPC * D                # q features per core: 256
P = 128
KT = E // P                  # 16 contraction tiles over E
NT = S // 512                # 4 token 512-blocks per batch
SCALE = D ** -0.5
F32 = mybir.dt.float32
BF16 = mybir.dt.bfloat16
FA = mybir.ActivationFunctionType
ALU = mybir.AluOpType


def build_nc():
    nc = bacc.Bacc()
    # x_t tiled [B, NT, E, 512]: each (b, n) token-block is a dense 2MB
    # region so the strided per-partition DMA rows stay page-local
    x_t = nc.declare_dram_parameter("x_t", [B, NT, E, 512], BF16, isOutput=False)
    wq = nc.declare_dram_parameter("wq", [E, FPC], BF16, isOutput=False)
    wkv = nc.declare_dram_parameter("wkv", [E, P], BF16, isOutput=False)
    wo = nc.declare_dram_parameter("wo", [FPC, E], BF16, isOutput=False)
    # out tiled [B, KT, NT, 128, 512]: every output DMA is one contiguous
    # 128KB block; the host reassembles
    out = nc.declare_dram_parameter("out", [B, KT, NT, P, 512], BF16, isOutput=True)

    with ExitStack() as ctx:
        tc = ctx.enter_context(tile.TileContext(nc))
        consts = ctx.enter_context(tc.tile_pool(name="consts", bufs=1))
        wpool = ctx.enter_context(tc.tile_pool(name="w", bufs=1))
        xbp = ctx.enter_context(tc.tile_pool(name="xb", bufs=4))
        qkvp = ctx.enter_context(tc.tile_pool(name="qkv", bufs=2))
        ppool = ctx.enter_context(tc.tile_pool(name="probs", bufs=14))
        npool = ctx.enter_context(tc.tile_pool(name="norm", bufs=2))
        opool = ctx.enter_context(tc.tile_pool(name="outsb", bufs=4))
        pp_mm = ctx.enter_context(tc.tile_pool(name="pmm", bufs=2, space="PSUM"))
        pp_sp = ctx.enter_context(tc.tile_pool(name="psp", bufs=2, space="PSUM"))
        pp_acc = ctx.enter_context(tc.tile_pool(name="pacc", bufs=2, space="PSUM"))

        # ---- constants ----
        ident = consts.tile([P, P], BF16)
        make_identity(nc, ident)
        tri = consts.tile([P, P], BF16)  # tri[k, q] = 1 iff q >= k
        make_upper_triangular(nc, tri, val=1.0, diag=True)
        # sel2 row 0 selects partitions 0-63, row 32 selects 64-127 (rows are
        # 32-aligned — the engines reject other base partitions). lhsT of the
        # K=64 broadcast matmul that fans the per-head 1/denom rows out to the
        # 64-partition feature blocks of attnsb.
        sel2 = consts.tile([64, P], BF16)
        nc.gpsimd.memset(sel2, 0.0)
        nc.gpsimd.memset(sel2[0:1, 0:64], 1.0)
        nc.gpsimd.memset(sel2[32:33, 64:128], 1.0)

        # ---- weights (gpsimd DMA queue, parallel to the x loads on sync) ----
        # wq in 4 ko-chunks so the opening q-proj matmuls only wait on the
        # first 256KB instead of the full megabyte
        wq_sb = wpool.tile([P, KT, FPC], BF16)
        wq_r = wq.rearrange("(ko p) m -> p ko m", p=P)
        for ch in range(4):
            nc.gpsimd.dma_start(wq_sb[:, ds(4 * ch, 4), :], wq_r[:, ds(4 * ch, 4), :])
        wkv_sb = wpool.tile([P, KT, P], BF16)
        nc.gpsimd.dma_start(wkv_sb, wkv.rearrange("(ko p) m -> p ko m", p=P))
        wo_sb = wpool.tile([P, 2, E], BF16)
        nc.gpsimd.dma_start(wo_sb, wo.rearrange("(ko p) m -> p ko m", p=P))

        state = {("normed", 0): [0], ("normed", 1): [0]}

        def gen_proj_n(b, n):
            """projection of token-block n for batch b (PE-heavy)."""
            if n == 0:
                # q4: per-head q^T at partitions 0-63, zeros at 64-127 so the
                # scores matmul contracts over a full 128 partitions (keeps
                # the PE in one tile mode — no mode-switch drains).
                q4 = qkvp.tile([P, HPC, S], BF16, tag="q4")
                nc.gpsimd.memset(q4[64:128, :, :], 0.0)
                # k_pad: k^T rows 0-63, zeros 64-127 (scores lhsT)
                k_pad = qkvp.tile([P, S], BF16, tag="kpad")
                nc.gpsimd.memset(k_pad[64:128, :], 0.0)
                # kv: k^T rows 0-63, v^T rows 64-127 (v-transpose source)
                kvsb = qkvp.tile([P, S], BF16, tag="kv")
                # v token-major (+ ones column), PE-transposed per block
                vsb = qkvp.tile([P, S // P, D + 1], BF16, tag="v")
                nc.gpsimd.memset(vsb[:, :, D : D + 1], 1.0)
                # attnsb holds UNNORMALIZED attn until deferred normalize
                attnsb = qkvp.tile([P, 2, S], BF16, tag="attn")
                state[b] = (q4, k_pad, kvsb, vsb, attnsb)
            q4, k_pad, kvsb, vsb, attnsb = state[b]
            # two half-tiles with separate DMAs: the first 8 k-tiles of
            # matmul only wait on the first half's DMA. The very first
            # block is DMA'd in quarters so the opening matmul can start
            # as early as possible.
            xh = []
            nchunk = 4 if (b == 0 and n == 0) else 1
            # very first block: spread the quarter DMAs across the two
            # hwdge engine queues so the transfers run in parallel instead
            # of serializing on the sync queue
            qeng = [nc.sync, nc.scalar, nc.sync, nc.scalar]
            for g in range(2):
                xb = xbp.tile([P, KT // 2, 512], BF16)
                for q_ in range(nchunk):
                    eng = qeng[q_] if nchunk > 1 else nc.sync
                    eng.dma_start(
                        xb[:, ds(q_ * (8 // nchunk), 8 // nchunk), :],
                        x_t[
                            b, n,
                            ds(1024 * g + q_ * (1024 // nchunk), 1024 // nchunk),
                            :,
                        ].rearrange("(ko p) s -> p ko s", p=P),
                    )
                xh.append(xb)
            for m in range(3):
                ps = pp_mm.tile([P, 512], F32, tag="mm")
                for k in range(KT):
                    lhsT = wq_sb[:, k, ts(m, P)] if m < 2 else wkv_sb[:, k, :]
                    nc.tensor.matmul(
                        ps,
                        lhsT,
                        xh[k // 8][:, k % 8, :],
                        start=(k == 0),
                        stop=(k == KT - 1),
                    )
                if m < 2:
                    nc.vector.tensor_copy(q4[0:64, 2 * m, ts(n, 512)], ps[0:64, :])
                    nc.vector.tensor_copy(
                        q4[0:64, 2 * m + 1, ts(n, 512)], ps[64:128, :]
                    )
                else:
                    nc.vector.tensor_copy(kvsb[:, ts(n, 512)], ps)
                    nc.vector.tensor_copy(k_pad[0:64, ts(n, 512)], ps[0:64, :])
                yield
            # v transposes for this token-block, done per-block so a proj
            # generator that finishes block n has also produced vsb block n —
            # lets proj(1)'s later blocks run as phase-3 filler without the
            # attention racing ahead of its vsb inputs in the PE queue
            for st in range(4 * n, 4 * n + 4):
                tp = pp_mm.tile([P, P], BF16, tag="mm")
                nc.tensor.transpose(tp, kvsb[:, ts(st, P)], ident)
                nc.vector.tensor_copy(vsb[:, st, 0:D], tp[:, 64:128])
            yield

        def gen_att_qt(b, qt):
            """causal attention for q-block qt of batch b (ACT-heavy: exp).

            softmax denominators come for free from the ones-column of v_aug
            (row 64 of each accumulator); per-head approximate reciprocals
            run straight from PSUM, and the rest of the normalization (bf16
            cast + selector-matmul broadcast + multiplies) is DEFERRED into
            the next q-block's instruction stream (gen_norm), so the bc
            matmul never stalls the in-order PE queue waiting on the DVE
            reciprocal chain.
            """
            q4, k_pad, kvsb, vsb, attnsb = state[b]
            if True:
                # dq[32*(h%2), 512*(h//2) + q] = denominator of head h at
                # query q (rows 32-aligned). The 1.0 background keeps the
                # reciprocal finite on the unused rows so the selector matmul
                # contracts only finite values. rec/recb are filled per
                # half (after heads 1 and 3) so the bc matmul's input is
                # ready long before the deferred normalize drains.
                dq = npool.tile([64, 1024], F32, tag="den")
                nc.gpsimd.memset(dq, 1.0)
                rec = npool.tile([64, 1024], F32, tag="rec")
                recb = npool.tile([64, 1024], BF16, tag="recb")
                nkt = 4 * (qt + 1)
                for h in range(HPC):
                    acc = pp_acc.tile([D + 1, 512], F32, tag="acc")
                    nfull = 4 * qt
                    # 1) diagonal scores first: their exp+mask chains get
                    #    maximum slack before their attnV consumers issue
                    #    last. t=0 gets its own PSUM tile; t=1..3 pack into a
                    #    second one (bank-aligned at 0/512/768) so ONE wide
                    #    exp covers all three — fewer ACTIVATEs on the
                    #    engine that paces the attention phase. The packed
                    #    ACTIVATE also exps the stale gap cols 384..512;
                    #    harmless (scaled stale PSUM stays well below f32
                    #    overflow, and no consumer reads those pr cols).
                    dof = {1: 0, 2: 512, 3: 768}
                    spA = pp_sp.tile([P, 1024], F32, tag="sp")
                    nc.tensor.matmul(
                        spA[:, 0:512],
                        k_pad[:, ts(nfull, P)],
                        q4[:, h, ts(qt, 512)],
                        start=True,
                        stop=True,
                    )
                    prA = ppool.tile([P, 1024], BF16, tag="pr")
                    nc.scalar.activation(
                        prA[:, 0:512], spA[:, 0:512], FA.Exp, scale=SCALE
                    )
                    nc.vector.tensor_tensor(
                        prA[:, 0:P], prA[:, 0:P], tri, ALU.mult
                    )
                    spB = pp_sp.tile([P, 1024], F32, tag="sp")
                    for t in range(1, 4):
                        kt = nfull + t
                        w_ = 512 - t * P
                        nc.tensor.matmul(
                            spB[:, ds(dof[t], w_)],
                            k_pad[:, ts(kt, P)],
                            q4[:, h, ds(512 * qt + t * P, w_)],
                            start=True,
                            stop=True,
                        )
                    prB = ppool.tile([P, 1024], BF16, tag="pr")
                    nc.scalar.activation(
                        prB[:, 0:896], spB[:, 0:896], FA.Exp, scale=SCALE
                    )
                    for t in range(1, 4):
                        nc.vector.tensor_tensor(
                            prB[:, ds(dof[t], P)], prB[:, ds(dof[t], P)],
                            tri, ALU.mult,
                        )
                    yield
                    # 2) full (off-diagonal) k-tiles, paired two per PSUM tile
                    #    so one exp ACTIVATE covers 1024 columns; attnV follows
                    #    each pair immediately (no mask on its path)
                    first = True
                    for kp in range(0, nfull, 2):
                        sp = pp_sp.tile([P, 1024], F32, tag="sp")
                        for j in range(2):
                            kt = kp + j
                            nc.tensor.matmul(
                                sp[:, ts(j, 512)],
                                k_pad[:, ts(kt, P)],
                                q4[:, h, ts(qt, 512)],
                                start=True,
                                stop=True,
                            )
                        pr = ppool.tile([P, 1024], BF16, tag="pr")
                        nc.scalar.activation(pr, sp, FA.Exp, scale=SCALE)
                        for j in range(2):
                            nc.tensor.matmul(
                                acc,
                                vsb[:, kp + j, :],
                                pr[:, ts(j, 512)],
                                start=first,
                                stop=False,
                            )
                            first = False
                        if kp % 4 == 2:
                            yield
                    # 3) diagonal attnV last
                    for t in range(4):
                        kt = nfull + t
                        off = t * P
                        w_ = 512 - off
                        src = prA[:, ds(off, w_)] if t == 0 else prB[:, ds(dof[t], w_)]
                        nc.tensor.matmul(
                            acc[:, ds(off, w_)],
                            vsb[:, kt, :],
                            src,
                            start=first,
                            stop=(t == 3),
                        )
                        first = False
                    yield
                    # evacuate unnormalized attn + denominator row (head h's
                    # denominators land at partition 32(h%2) for broadcast
                    # later). Denominator row first: it gates the deferred
                    # normalize chain.
                    dst = attnsb[64 * (h % 2) : 64 * (h % 2) + 64, h // 2, ts(qt, 512)]
                    nc.vector.tensor_copy(
                        dq[32 * (h % 2) : 32 * (h % 2) + 1, ds(512 * (h // 2), 512)],
                        acc[D : D + 1, :],
                    )
                    nc.vector.tensor_copy(dst, acc[0:D, :])
                    if h % 2 == 1:
                        # both rows of this half's denominators are in: fold
                        # them to bf16 reciprocals now, off the critical path
                        half = ds(512 * (h // 2), 512)
                        nc.vector.reciprocal_approx_fast(rec[:, half], dq[:, half])
                        nc.vector.tensor_copy(recb[:, half], rec[:, half])
                    yield
                state[("pending_norm", b)] = gen_norm(b, qt, recb)

        def gen_norm(b, qt, recb):
            """deferred softmax normalization for q-block qt: one K=64
            selector matmul (2 PSUM-bank halves) broadcasts both feature
            blocks' 1/denom rows across partitions, two multiplies apply
            them. Drained several steps into the NEXT q-block so the bc
            matmul queues behind fresh PE work while the reciprocal chain
            (already emitted per-half) drains."""
            attnsb = state[b][4]
            bc = pp_sp.tile([P, 1024], F32, tag="sp")
            for kk in range(2):
                nc.tensor.matmul(
                    bc[:, ts(kk, 512)], sel2, recb[:, ts(kk, 512)],
                    start=True, stop=True,
                )
            state[("normed", b)][0] += 1
            yield
            for kk in range(2):
                dst = attnsb[:, kk, ts(qt, 512)]
                nc.vector.tensor_tensor(dst, dst, bc[:, ts(kk, 512)], ALU.mult)
            yield

        def gen_outproj(b, use_act, ns=None):
            """partial output projection for batch b (PE-heavy).

            n-outer so the last q-block's softmax-normalize latency is hidden
            behind the first 3 n-blocks' matmuls. use_act alternates the PSUM
            evacuation onto ScalarE only when no attention phase is keeping
            ScalarE saturated with exps.

            Gated on the normalize counter: the normalize path now contains a
            PE matmul (the 1/denom broadcast), so emitting an outproj matmul
            that waits on q-block n's normalize BEFORE that broadcast matmul
            is emitted would deadlock the in-order PE queue.
            """
            attnsb = state[b][4]
            for n in ns if ns is not None else range(NT):
                while state[("normed", b)][0] <= n:
                    yield
                for m in range(KT):
                    po = pp_mm.tile([P, 512], F32, tag="mm")
                    for kk in range(2):
                        nc.tensor.matmul(
                            po,
                            wo_sb[:, kk, ts(m, P)],
                            attnsb[:, kk, ts(n, 512)],
                            start=(kk == 0),
                            stop=(kk == 1),
                        )
                    osb = opool.tile([P, 512], BF16)
                    # use_act None: never touch ScalarE (it paces the
                    # attention phase this generator fills); True: quiet-tail
                    # block, lean on ScalarE to relieve DVE
                    if use_act is True and m % 3 != 0:
                        nc.scalar.copy(osb, po)
                    elif use_act is False and m % 3 == 2:
                        nc.scalar.copy(osb, po)
                    else:
                        nc.vector.tensor_copy(osb, po)
                    # final block: alternate DMA queues so the closing
                    # transfers drain in parallel instead of serializing on
                    # the sync queue after the last matmul
                    deng = nc.scalar if (b == 1 and n == 3 and m % 2 == 1) else nc.sync
                    deng.dma_start(out[b, m, n, :, :], osb)
                    if m % 4 == 3:
                        yield

        def run_all(gen):
            for _ in gen:
                pass

        def interleave(pairs):
            """pairs: list of [gen, steps_per_round]. Round-robin with ratios
            so the PE-filler generator is spread across the whole phase."""
            pairs = [[g, r] for g, r in pairs]
            while pairs:
                for gr in pairs[:]:
                    try:
                        for _ in range(gr[1]):
                            next(gr[0])
                    except StopIteration:
                        pairs.remove(gr)

        def delayed(gen, k):
            for _ in range(k):
                yield
            yield from gen

        def chain(gens):
            for g in gens:
                yield from g

        def att_batch(b):
            """attention for all q-blocks of batch b, draining each block's
            deferred normalize a few steps into the NEXT block (so the bc
            matmul hides behind fresh scores/attnV work), and the final one
            with interleaver turns between its pieces."""
            for qt in range(NT):
                g = gen_att_qt(b, qt)
                steps = 0
                for _ in g:
                    yield
                    steps += 1
                    if steps == 8 and ("pending_norm", b) in state:
                        for _ in state.pop(("pending_norm", b)):
                            yield
            for _ in range(6):
                yield
            for _ in state.pop(("pending_norm", b)):
                yield

        # Pipeline the two batches so PE-heavy projection work fills the PE
        # bubbles of the ACT(exp)-bound attention phases; out-projections
        # enter a phase early, delayed so their first matmuls trail the
        # q-block normalizes they depend on in the in-order PE stream.
        # op0 is split: a delayed sliver covers the tail of the batch-0
        # attention phase (after proj(1) exhausts), the bulk fills the
        # batch-1 attention phase's PE bubbles.
        op0a = gen_outproj(0, None, ns=[0])
        op0b = gen_outproj(0, None, ns=[1, 2, 3])
        # both ops' bulk runs inside exp-saturated attention phases -> keep
        # their evacuations off the scalar engine entirely; only the final
        # n=3 block (the quiet tail) borrows ScalarE
        op1 = gen_outproj(1, None, ns=[0, 1, 2])
        op1t = gen_outproj(1, True, ns=[3])
        proj = lambda b: chain([gen_proj_n(b, n) for n in range(NT)])
        run_all(proj(0))
        interleave([(att_batch(0), 4), (proj(1), 1), (delayed(op0a, 13), 1)])
        interleave([(op0b, 1), (att_batch(1), 4), (op1, 1)])
        run_all(op1)
        run_all(op1t)
    return nc


BF = ml_dtypes.bfloat16


def make_in_maps(x, Wq, Wk, Wv, Wo):
    # [B, S, E] -> [B, NT, E, 512] (token-block-tiled, feature-major)
    x_t = np.ascontiguousarray(
        np.transpose(
            np.asarray(x, np.float32).reshape(B, NT, 512, E), (0, 1, 3, 2)
        )
    ).astype(BF)
    Wq = np.asarray(Wq, np.float32)
    Wk = np.asarray(Wk, np.float32)
    Wv = np.asarray(Wv, np.float32)
    Wo = np.asarray(Wo, np.float32)
    in_maps = []
    for c in range(NCORES):
        wq_sh = np.ascontiguousarray(Wq[:, FPC * c : FPC * (c + 1)]).astype(BF)
        wkv_sh = np.concatenate(
            [Wk[:, D * c : D * (c + 1)], Wv[:, D * c : D * (c + 1)]], axis=1
        ).astype(BF)
        wo_sh = np.ascontiguousarray(Wo[FPC * c : FPC * (c + 1), :]).astype(BF)
        in_maps.append({"x_t": x_t, "wq": wq_sh, "wkv": wkv_sh, "wo": wo_sh})
    return in_maps


_NC_CACHE = {}


def get_nc():
    if "nc" not in _NC_CACHE:
        nc = build_nc()
        nc.compile()
        _NC_CACHE["nc"] = nc
    return _NC_CACHE["nc"]


def kernel(x, Wq, Wk, Wv, Wo, bo, mask=None, **_ignored):
    nc = get_nc()
    in_maps = make_in_maps(x, Wq, Wk, Wv, Wo)
    res = run_bass_kernel_spmd(nc, in_maps, list(range(NCORES)))
    total = np.zeros((B, KT, NT, P, 512), np.float32)
    for c in range(NCORES):
        total += np.asarray(res.results[c]["out"], np.float32)
    # [B, KT, NT, 128, 512] -> [B, S, E]: feature = m*128+p, token = n*512+s
    full = np.transpose(total, (0, 2, 4, 1, 3)).reshape(B, S, E)
    full = full + np.asarray(bo, np.float32)[None, None, :]
    return np.ascontiguousarray(full)

